# revision 3
# baseline (speedup 1.0000x reference)
"""Caser (dense_cnn) Trainium2 kernel: batch-parallel over 8 NeuronCores.

Strategy
--------
Data-parallel over batch (512 rows/core); conv/fc weights folded on host
into two matmuls; item/user/W2 tables replicated per core and gathered
on-device with SWDGE indirect DMA (one row per partition per instruction;
the BIR lowering only supports a 2D dynamic AP, so 128 rows/instruction
is the max batch).

Per core:
  stage 1: gather item embeddings + user embeddings (the latter straight
           into xcaug[:, D:2D]), PE-transpose, two folded matmuls
           -> xcaug = [relu(fc1) | uemb | 1.0 | 0] f32, then a scaled f16
           copy (x64, Act engine) for stage 2
  stage 2: gather w2cat rows (w2|b2|0, float16 pre-scaled x64 on host)
           per (b, t); VectorE broadcast-mult (f16) + one full-row
           tensor_reduce to f32 (covers dot AND b2 in one op)
           -> res[b, t] scaled by 4096; one fused result DMA per core;
           host divides by 4096 (exact).

W2 is float16 (10 mantissa bits): measured 2.9e-3 absmax/std vs the 2e-2
gate; the x64 power-of-2 pre-scales keep every product in f16 normal
range. f16 halves the dominant gather bytes (1032B -> 516B rows).

Gather stream tuning (all measured on-device via reps-in-NEFF median
wall-clock slope with device-resident inputs):
  - 2D out AP (128 rows/instruction) streams at ~1.6 us/instruction
    with the default flags; single_packet=True cuts it to ~1.28;
    oob_is_err=False to ~1.0; num_swdge_queues=2 (round-robin) to
    ~0.93 us/instruction (7.3 ns/row, ~71 GB/s effective).
  - 48KB dynamic-DMA descriptor carveout beats 16/32/64/96KB.
  - Gather time is instruction-rate-bound, not byte-bound (258 vs 129
    f16 columns time the same), so desc-gen on the (single, pair-0) Q7
    is the serial resource: 424 gather instructions/core ~= 400 us.
All DVE/PE/Act work and the DMA transfers hide under the gather stream.
"""

import numpy as np

from concourse import bass, bacc, mybir, tile
from concourse.bass_utils import run_bass_kernel_spmd
from concourse.masks import make_identity

B, L, D, NH, NV, T = 4096, 5, 128, 16, 4, 100
NITEMS = NUSERS = 100000
NCORES = 8
BC = B // NCORES
NBLK = BC // 128
ROW = 2 * D + 2
FC1 = 752
HOR = 240
XW = D + HOR

F32 = mybir.dt.float32
F16 = mybir.dt.float16
I32 = mybir.dt.int32
SW2 = 64.0   # w2cat pre-scale (exact power of 2; keeps f16 products in normal range)
SXC = 64.0   # xcaug pre-scale; host divides results by SW2*SXC
NQ = 2       # SWDGE queue rings; indirect gathers round-robin across them
SCRATCH = 49152


def _indirect_gather(nc, out, in_, offset_ap, queue_num):
    """nc.gpsimd indirect InstDMACopy (src-indirect, axis 0), 128 rows.

    single_packet=True and oob_is_err=False each measurably speed up the
    Q7 descriptor-generation loop (1.28 -> 1.0 us/instruction and
    1.61 -> 1.28 respectively at 258 f16 columns).
    """
    eng = nc.gpsimd
    src_ap = in_
    assert isinstance(src_ap.offset, int) and src_ap.offset == 0
    out_l = eng.lower_ap_dma(out, for_indirect_dma=True)
    in_l = eng.lower_ap_dma(in_, for_indirect_dma=True)
    assert len(in_l) == 1 and len(out_l) == 1
    off_l = eng.lower_ap_dma(offset_ap)
    assert len(off_l) == 1
    in_l.append(off_l[0])
    coef = 1
    for i in range(1, len(src_ap.shape)):
        coef *= src_ap.shape[i]
    in_l[0].dynamic_ap_info = mybir.DynamicAccessPatternInfo(
        c=0,
        actual_ap=out.ap,
        indirect_dim_max_index=src_ap.shape[0],
        offset_expr=[
            mybir.DynamicAccessPatternOffsetExpr(
                coef=coef,
                aff_expr=mybir.DynamicAccessPatternOffsetExprAffExpr(
                    kind="IndirectArgId", arg_id=1
                ),
            )
        ],
    )
    return eng.add_instruction(
        mybir.InstDMACopy(
            name=nc.get_next_instruction_name(),
            queue=f"qPoolDynamic{queue_num or ''}",
            mode="Copy",
            ins=in_l,
            outs=out_l,
            oob_is_err=False,
            cce_op=mybir.AluOpType.bypass,
            single_packet=True,
        )
    )


def _build(tch=50, gbufs=4, reps=1):
    nch = T // tch
    nc = bacc.Bacc(None, target_bir_lowering=False, num_swdge_queues=NQ,
                   dynamic_dma_scratch_size=SCRATCH)

    w2cat = nc.declare_dram_parameter("w2cat", [NITEMS, ROW], F16, isOutput=False)
    item_t = nc.declare_dram_parameter("item_table", [NITEMS, D], F32, isOutput=False)
    user_t = nc.declare_dram_parameter("user_table", [NUSERS, D], F32, isOutput=False)
    seq_idx = nc.declare_dram_parameter("seq_idx", [128, NBLK * L], I32, isOutput=False)
    user_idx = nc.declare_dram_parameter("user_idx", [128, NBLK], I32, isOutput=False)
    items_idx = nc.declare_dram_parameter("items_idx", [128, NBLK * T], I32, isOutput=False)
    cc_d = nc.declare_dram_parameter("cc", [128, L, XW], F32, isOutput=False)
    w1b_d = nc.declare_dram_parameter("w1b", [120, 2, D], F32, isOutput=False)
    crow_d = nc.declare_dram_parameter("crow", [1, XW], F32, isOutput=False)
    res_d = nc.declare_dram_parameter("res", [BC, T], F32, isOutput=True)

    with tile.TileContext(nc) as tc:
        with (
            tc.tile_pool(name="const", bufs=1) as cp,
            tc.tile_pool(name="work", bufs=2) as wp,
            tc.tile_pool(name="gather", bufs=gbufs) as gp,
            tc.tile_pool(name="psum", bufs=2, space="PSUM") as pp,
        ):
            ident = cp.tile([128, 128], F32)
            make_identity(nc, ident[:])
            ones = cp.tile([1, 128], F32)
            nc.vector.memset(ones[:], 1.0)

            cc_sb = cp.tile([128, L, XW], F32)
            nc.sync.dma_start(out=cc_sb[:], in_=cc_d[:])
            w1b_sb = cp.tile([120, 2, D], F32)
            nc.sync.dma_start(out=w1b_sb[:], in_=w1b_d[:])
            crow_sb = cp.tile([1, XW], F32)
            nc.sync.dma_start(out=crow_sb[:], in_=crow_d[:])

            seqidx_sb = cp.tile([128, NBLK * L], I32)
            nc.sync.dma_start(out=seqidx_sb[:], in_=seq_idx[:])
            useridx_sb = cp.tile([128, NBLK], I32)
            nc.sync.dma_start(out=useridx_sb[:], in_=user_idx[:])
            itemsidx_sb = cp.tile([128, NBLK * T], I32)
            nc.sync.dma_start(out=itemsidx_sb[:], in_=items_idx[:])

            emb_sb = cp.tile([128, NBLK * L, D], F32)
            xcaug = [
                cp.tile([128, ROW], F32, tag=f"xcaug{b}", name=f"xcaug{b}")
                for b in range(NBLK)
            ]
            xch = [
                cp.tile([128, ROW], F16, tag=f"xch{b}", name=f"xch{b}")
                for b in range(NBLK)
            ]
            res_all = cp.tile([128, NBLK, T], F32)

            # xcaug trailing constants: [1.0 | 0.0], set once
            for blk in range(NBLK):
                nc.vector.memset(xcaug[blk][:, 2 * D : 2 * D + 1], 1.0)
                nc.vector.memset(xcaug[blk][:, 2 * D + 1 : ROW], 0.0)

            qn = [0]

            def next_q():
                q = qn[0] % NQ
                qn[0] += 1
                return q

            for rep in range(reps):
                # --- embedding gathers ---
                for j in range(NBLK * L):
                    _indirect_gather(
                        nc, emb_sb[:, j, :], item_t[:],
                        seqidx_sb[:, j : j + 1], next_q(),
                    )
                for j in range(NBLK):
                    # user embedding lands directly in xcaug[:, D:2D]
                    _indirect_gather(
                        nc, xcaug[j][:, D : 2 * D], user_t[:],
                        useridx_sb[:, j : j + 1], next_q(),
                    )

                # ---------------- stage 1 ----------------
                for blk in range(NBLK):
                    embT_ps = pp.tile([128, L * 128], F32, tag="embT")
                    for t in range(L):
                        nc.tensor.transpose(
                            out=embT_ps[:, t * 128 : (t + 1) * 128],
                            in_=emb_sb[:, blk * L + t, :],
                            identity=ident[:],
                        )
                    embT_sb = wp.tile([128, L * 128], F32, tag="embT_sb")
                    nc.scalar.copy(out=embT_sb[:], in_=embT_ps[:])

                    x_ps = pp.tile([128, XW], F32, tag="xps")
                    for t in range(L):
                        nc.tensor.matmul(
                            out=x_ps[:],
                            lhsT=embT_sb[:, t * 128 : (t + 1) * 128],
                            rhs=cc_sb[:, t, :],
                            start=(t == 0),
                            stop=False,
                            skip_group_check=True,
                        )
                    nc.tensor.matmul(
                        out=x_ps[:],
                        lhsT=ones[0:1, :],
                        rhs=crow_sb[0:1, :],
                        start=False,
                        stop=False,
                        skip_group_check=True,
                    )
                    h_sb = wp.tile([128, HOR], F32, tag="h_sb")
                    nc.scalar.activation(
                        out=h_sb[:],
                        in_=x_ps[:, D : D + HOR],
                        func=mybir.ActivationFunctionType.Relu,
                    )
                    hT_ps = pp.tile([120, 256], F32, tag="hT")
                    for c2 in range(2):
                        nc.tensor.transpose(
                            out=hT_ps[:, c2 * 128 : c2 * 128 + 128],
                            in_=h_sb[:, c2 * 120 : (c2 + 1) * 120],
                            identity=ident[:],
                        )
                    hT_sb = wp.tile([120, 256], F32, tag="hT_sb")
                    nc.scalar.copy(out=hT_sb[:], in_=hT_ps[:])
                    for c2 in range(2):
                        nc.tensor.matmul(
                            out=x_ps[:, 0:D],
                            lhsT=hT_sb[:, c2 * 128 : c2 * 128 + 128],
                            rhs=w1b_sb[:, c2, :],
                            start=False,
                            stop=(c2 == 1),
                            skip_group_check=True,
                        )
                    nc.scalar.activation(
                        out=xcaug[blk][:, 0:D],
                        in_=x_ps[:, 0:D],
                        func=mybir.ActivationFunctionType.Relu,
                    )
                    # scaled f16 copy for stage 2 (Act engine, off the gather path)
                    nc.scalar.mul(out=xch[blk][:], in_=xcaug[blk][:], mul=SXC)

                # ---------------- stage 2 ----------------
                for blk in range(NBLK):
                    for h in range(nch):
                        c0 = blk * T + h * tch
                        w2_sb = gp.tile([128, tch, ROW], F16, tag="w2")
                        for t in range(tch):
                            _indirect_gather(
                                nc, w2_sb[:, t, :], w2cat[:],
                                itemsidx_sb[:, c0 + t : c0 + t + 1],
                                next_q(),
                            )
                        xa = xch[blk][:]
                        xb = bass.AP(xa.tensor, xa.offset, [xa.ap[0], [0, tch], [1, ROW]])
                        nc.vector.tensor_tensor(
                            out=w2_sb[:], in0=w2_sb[:], in1=xb, op=mybir.AluOpType.mult
                        )
                        # one-pass reduce over the FULL row: cols 0:256 are
                        # w2*xc products, col 256 is b2*SW2*SXC, col 257 is 0,
                        # so the sum is the complete (scaled) score.
                        nc.vector.tensor_reduce(
                            out=res_all[:, blk, h * tch : (h + 1) * tch],
                            in_=w2_sb[:],
                            axis=mybir.AxisListType.X,
                            op=mybir.AluOpType.add,
                        )
                    # per-blk result store as soon as its last reduce lands:
                    # res_d[blk*128+p, t] = res_all[p, blk, t]
                    nc.sync.dma_start(
                        out=res_d[blk * 128 : (blk + 1) * 128, :],
                        in_=res_all[:, blk, :],
                    )

    nc.finalize()
    return nc


def _fold_weights(Wv, bv, Wh, bh, W1, b1):
    Wv = np.asarray(Wv, np.float32)
    bv = np.asarray(bv, np.float32)
    Wh = np.asarray(Wh, np.float32)
    bh = np.asarray(bh, np.float32)
    W1 = np.asarray(W1, np.float32)
    b1 = np.asarray(b1, np.float32)

    W1a = W1[: NV * D].reshape(NV, D, D)
    C = np.einsum("vt,vdm->tdm", Wv, W1a)
    const_bv = np.einsum("v,vdm->m", bv, W1a)

    H = np.zeros((L, D, HOR), np.float32)
    bh_rep = np.zeros(HOR, np.float32)
    off = 0
    for l in range(1, L + 1):
        lout = L - l + 1
        for f in range(NH):
            for tau in range(lout):
                j = off + f * lout + tau
                bh_rep[j] = bh[l - 1, f]
                for s in range(l):
                    H[tau + s, :, j] = Wh[l - 1, f, s, :]
        off += NH * lout

    CC = np.concatenate([C.reshape(L * D, D), H.reshape(L * D, HOR)], axis=1)
    cc = CC.reshape(L, 128, XW).transpose(1, 0, 2).copy()

    W1b = W1[NV * D :]
    w1b = W1b.reshape(2, 120, D).transpose(1, 0, 2).copy()

    crow = np.zeros((1, XW), np.float32)
    crow[0, :D] = b1 + const_bv
    crow[0, D:] = bh_rep
    return cc, w1b, crow


def _prepare(inputs):
    seq = np.asarray(inputs["seq"]).astype(np.int32)
    user = np.asarray(inputs["user"]).astype(np.int32)
    items = np.asarray(inputs["items"]).astype(np.int32)
    item_table = np.asarray(inputs["item_table"], np.float32)
    user_table = np.asarray(inputs["user_table"], np.float32)
    W2_table = np.asarray(inputs["W2_table"], np.float32)
    b2_table = np.asarray(inputs["b2_table"], np.float32)

    cc, w1b, crow = _fold_weights(
        inputs["Wv"], inputs["bv"], inputs["Wh"], inputs["bh"],
        inputs["W1"], inputs["b1"],
    )

    w2cat = np.zeros((NITEMS, ROW), np.float32)
    w2cat[:, : 2 * D] = W2_table
    w2cat[:, 2 * D] = b2_table[:, 0]
    w2cat = (w2cat * SW2).astype(np.float16)

    in_maps = []
    for c in range(NCORES):
        s = slice(c * BC, (c + 1) * BC)
        sq = seq[s].reshape(NBLK, 128, L).transpose(1, 0, 2).reshape(128, NBLK * L)
        us = user[s].reshape(NBLK, 128).transpose(1, 0).copy()
        it = items[s].reshape(NBLK, 128, T).transpose(1, 0, 2).reshape(128, NBLK * T)
        in_maps.append(
            {
                "w2cat": w2cat,
                "item_table": item_table,
                "user_table": user_table,
                "seq_idx": np.ascontiguousarray(sq),
                "user_idx": np.ascontiguousarray(us),
                "items_idx": np.ascontiguousarray(it),
                "cc": cc,
                "w1b": w1b,
                "crow": crow,
            }
        )
    return in_maps


_NC = None


def kernel(**inputs):
    global _NC
    if _NC is None:
        _NC = _build()
    in_maps = _prepare(inputs)
    r = run_bass_kernel_spmd(_NC, in_maps, list(range(NCORES)))
    out = np.concatenate([r.results[c]["res"] for c in range(NCORES)], axis=0)
    return (out * (1.0 / (SW2 * SXC))).astype(np.float32)
